# revision 1
# baseline (speedup 1.0000x reference)
"""ClusterGCN + 2x GAT message-passing kernel for 8 Trainium2 NeuronCores.

Strategy (dst-sharded, one SPMD program):
  - Nodes are permuted into 784 tiles of 128 slots, load-balanced so every
    tile has (nearly) the same number of incoming edges (self-loops added).
    Cores own 98 consecutive tiles each.
  - Per layer, each core gathers the rows of its incoming messages from a
    replicated node table in its HBM via batched indirect DMA, reduces them
    per dst tile with 0/1 selection-matrix matmuls accumulated in PSUM, and
    applies the layer transform in feature-major (transposed) space.
  - GAT softmax runs without max-subtraction (logits are small); per-edge
    attention scalars come from s_src packed in the gathered row (hi/lo bf16
    pair = ~16-bit mantissa) plus a batched 4-byte gather of local s_dst.
  - Between layers the per-core z-tables (h @ W with packed attention
    scalars) are AllGathered so every core can gather arbitrary source rows.
"""

import sys

sys.path.insert(0, "/opt/trn_rl_repo")

import numpy as np

import concourse.bacc as bacc
import concourse.bass as bass
import concourse.mybir as mybir
import concourse.tile as tile
from concourse.bass_utils import run_bass_kernel_spmd

# ---- problem constants (hardcoded per contest rules) ----
N = 100000
E = 1600000
FIN = 64
HID = 64
FOUT = 32
NEG = 0.2

P = 128
NCORES = 8
TILES_PER_CORE = 98
T_ALL = NCORES * TILES_PER_CORE  # 784
NPC = TILES_PER_CORE * P  # 12544 nodes per core
NP_ALL = T_ALL * P  # 100352 padded node count

FW1 = 68  # z1 row: z(64) | 1.0 | s_hi | s_lo | pad
FW2 = 36  # z2 row: z(32) | 1.0 | s_hi | s_lo | pad
BATCH = 7  # dst tiles per gather batch

F32 = mybir.dt.float32
BF16 = mybir.dt.bfloat16
I32 = mybir.dt.int32
AF = mybir.ActivationFunctionType
OP = mybir.AluOpType

_cache = {}
last_result = None


def _bf16(a):
    import ml_dtypes

    return np.asarray(a, dtype=ml_dtypes.bfloat16)


# ----------------------------------------------------------------------------
# host-side preprocessing
# ----------------------------------------------------------------------------
def _balance_tiles(deg):
    """Assign each of NP_ALL nodes to one of T_ALL tiles (128 slots each) so
    that per-tile total in-degree is near-uniform. Returns perm arrays."""
    import heapq

    order = np.argsort(-deg, kind="stable")
    heap = [(0, t) for t in range(T_ALL)]
    heapq.heapify(heap)
    counts = np.zeros(T_ALL, np.int64)
    loads = np.zeros(T_ALL, np.int64)
    tile_of = np.empty(NP_ALL, np.int32)
    slot_of = np.empty(NP_ALL, np.int32)
    for n in order:
        while True:
            load, t = heapq.heappop(heap)
            if counts[t] < P:
                break
        tile_of[n] = t
        slot_of[n] = counts[t]
        counts[t] += 1
        loads[t] += deg[n]
        if counts[t] < P:
            heapq.heappush(heap, (loads[t], t))
    return tile_of, slot_of, int(loads.max())


def _preprocess(x, edge_index):
    src = np.asarray(edge_index[0], np.int64)
    dst = np.asarray(edge_index[1], np.int64)
    loops = np.arange(NP_ALL, dtype=np.int64)
    src_all = np.concatenate([src, loops])
    dst_all = np.concatenate([dst, loops])
    deg = np.bincount(dst_all, minlength=NP_ALL)  # includes self-loop

    tile_of, slot_of, max_load = _balance_tiles(deg)
    ku = (max_load + P - 1) // P
    gid = tile_of.astype(np.int64) * P + slot_of  # node -> permuted row

    # per-message fields
    m_src = gid[src_all]  # gather row id
    m_tile = tile_of[dst_all].astype(np.int64)  # dst tile
    m_slot = slot_of[dst_all].astype(np.int64)  # dst slot in tile (0..127)
    # s_dst table layout per core: s[slot*98 + tile_local]
    m_sidx = m_slot * TILES_PER_CORE + (m_tile % TILES_PER_CORE)

    # bucket messages by tile, place message i of tile t at (p=i%128, c=i//128)
    order = np.argsort(m_tile, kind="stable")
    m_src, m_tile, m_slot, m_sidx = (
        m_src[order],
        m_tile[order],
        m_slot[order],
        m_sidx[order],
    )
    tile_counts = np.bincount(m_tile, minlength=T_ALL)
    tile_starts = np.concatenate([[0], np.cumsum(tile_counts)[:-1]])
    pos = np.arange(len(m_src)) - tile_starts[m_tile]  # rank within tile
    mp = pos % P
    mc = pos // P

    # padded per-core arrays [128, 98*ku]
    cols = TILES_PER_CORE * ku
    midx = np.zeros((NCORES, P, cols), np.int32)
    mdst = np.zeros((NCORES, P, cols), np.int32)
    mloc = np.full((NCORES, P, cols), -1.0, np.float32)
    core = m_tile // TILES_PER_CORE
    tl = m_tile % TILES_PER_CORE
    col = tl * ku + mc
    midx[core, mp, col] = m_src
    mdst[core, mp, col] = m_sidx
    mloc[core, mp, col] = m_slot

    deg_inv = (1.0 / np.maximum(deg, 1.0)).astype(np.float32)
    deginv_core = deg_inv[np.argsort(gid)].reshape(NCORES, TILES_PER_CORE, P)
    deginv_core = np.ascontiguousarray(np.transpose(deginv_core, (0, 2, 1)))

    # permuted node table
    inv = np.argsort(gid)  # permuted row -> original node
    xp = np.zeros((NP_ALL, FIN), np.float32)
    xv = np.asarray(x, np.float32)
    xp[gid[:N]] = xv[:N] if xv.shape[0] == N else xv
    return dict(
        ku=int(ku),
        midx=midx,
        mdst=mdst,
        mloc=mloc,
        deginv=deginv_core,
        xp=xp,
        inv=inv,
        gid=gid,
    )


# ----------------------------------------------------------------------------
# device program
# ----------------------------------------------------------------------------
def _padP(a):
    """pad first dim to P=128 with zeros"""
    out = np.zeros((P, a.shape[1]), a.dtype)
    out[: a.shape[0]] = a
    return out


def _hilo(v):
    hi = _bf16(np.asarray(v, np.float32))
    lo = _bf16(np.asarray(v, np.float32) - np.asarray(hi, np.float32))
    return hi, lo


def _build_program(ku):
    import os
    phases = int(os.environ.get("KERNEL_PHASES", "3"))
    nc = bacc.Bacc()
    cols = TILES_PER_CORE * ku

    # inputs (consts packed into 3 arrays to keep DMA sem fan-in small)
    CF = 555
    CB = P + cols + P
    CI = 2 * cols
    xtab = nc.declare_dram_parameter("xtab", [NP_ALL, FIN], BF16, isOutput=False)
    xloc = nc.declare_dram_parameter("xloc", [NPC, FIN], F32, isOutput=False)
    cf_in = nc.declare_dram_parameter("constf", [P, CF], F32, isOutput=False)
    cb_in = nc.declare_dram_parameter("constb", [P, CB], BF16, isOutput=False)
    ci_in = nc.declare_dram_parameter("consti", [P, CI], I32, isOutput=False)
    outloc = nc.declare_dram_parameter("outloc", [NPC, FOUT], F32, isOutput=True)

    # internal DRAM
    z1loc = nc.dram_tensor("z1loc", [NPC, FW1], BF16)
    z1tab = nc.dram_tensor("z1tab", [NP_ALL, FW1], BF16, addr_space="Shared")
    z2loc = nc.dram_tensor("z2loc", [NPC, FW2], BF16)
    z2tab = nc.dram_tensor("z2tab", [NP_ALL, FW2], BF16, addr_space="Shared")
    sd1 = nc.dram_tensor("sd1", [NPC, 1], F32)
    sd2 = nc.dram_tensor("sd2", [NPC, 1], F32)

    groups = [list(range(NCORES))]
    nb = (TILES_PER_CORE + BATCH - 1) // BATCH

    with tile.TileContext(nc) as tc:
        with (
            tc.tile_pool(name="const", bufs=1) as cpool,
            tc.tile_pool(name="sbuf", bufs=4) as pool,
            tc.tile_pool(name="gath", bufs=6) as gpool,
            tc.tile_pool(name="psum", bufs=2, space="PSUM") as pacc,
            tc.tile_pool(name="psum1", bufs=1, space="PSUM") as ptp,
        ):
            # ---- constants resident in SBUF ----
            def cload(ap, shape, dt, tag):
                t = cpool.tile(shape, dt, tag=tag)
                nc.sync.dma_start(out=t[:], in_=ap)
                return t

            cf = cload(cf_in[:, :], [P, CF], F32, tag="cf")
            cb = cload(cb_in[:, :], [P, CB], BF16, tag="cb")
            ci = cload(ci_in[:, :], [P, CI], I32, tag="ci")
            ident_t = cf[:, 0:128]
            dinv_t = cf[:, 128:226]
            b1r_t = cf[:, 226:290]
            b2r_t = cf[:, 290:322]
            bout_t = cf[:HID, 322:323]
            a1_t = cf[:HID, 323:327]
            a2_t = cf[:FOUT, 327:331]
            wout_t = cf[:FIN, 331:395]
            wroot_t = cf[:FIN, 395:459]
            w1_t = cf[:HID, 459:523]
            w2_t = cf[:HID, 523:555]
            iota_t = cb[:, 0:128]
            mloc_t = cb[:, 128 : 128 + cols]
            identb_t = cb[:, 128 + cols :]
            midx_t = ci[:, 0:cols]
            mdst_t = ci[:, cols:]

            sdcol = cpool.tile([P, TILES_PER_CORE], F32, tag="sdcol")

            def sel_build(ti):
                """0/1 bf16 selection [P, ku, P] for tile ti."""
                sel = pool.tile([P, ku, P], BF16, tag="sel")
                nc.vector.tensor_tensor(
                    out=sel[:, :, :],
                    in0=mloc_t[:, ti * ku : (ti + 1) * ku, None].to_broadcast(
                        [P, ku, P]
                    ),
                    in1=iota_t[:, None, :].to_broadcast([P, ku, P]),
                    op=OP.is_equal,
                )
                return sel

            def transform_and_pack(hin_sb, w_t, a_t, fi, fo, fw, zloc, ti):
                """Given node-major f32 activations hin_sb [P, fi] for tile ti:
                compute z = h @ W [P, fo] (via feature-major matmuls), s_src /
                s_dst = z @ a, write packed z-row to zloc, stash s_dst col.
                """
                # transpose h -> [fi, P]
                hT_ps = ptp.tile([fi, P], F32, tag="tp")
                nc.tensor.transpose(
                    out=hT_ps[:], in_=hin_sb[:, :], identity=ident_t
                )
                hT_sb = pool.tile([fi, P], F32, tag="hT")
                nc.vector.tensor_copy(out=hT_sb[:], in_=hT_ps[:])
                # z_T = W.T @ h_T  [fo, P]
                zT_ps = ptp.tile([fo, P], F32, tag="zT")
                nc.tensor.matmul(
                    out=zT_ps[:], lhsT=w_t, rhs=hT_sb[:, :], start=True, stop=True
                )
                zT_sb = pool.tile([fo, P], F32, tag="zTsb")
                nc.vector.tensor_copy(out=zT_sb[:], in_=zT_ps[:])
                # s columns: [P, 4] = z.T @ [a_src_hi a_src_lo a_dst_hi a_dst_lo]
                sc_ps = ptp.tile([P, 4], F32, tag="sc")
                nc.tensor.matmul(
                    out=sc_ps[:], lhsT=zT_sb[:, :], rhs=a_t, start=True, stop=True
                )
                sc_sb = pool.tile([P, 4], F32, tag="sc_sb")
                nc.vector.tensor_copy(out=sc_sb[:], in_=sc_ps[:, :])
                ssrc = pool.tile([P, 1], F32, tag="ssrc")
                nc.vector.tensor_tensor(
                    out=ssrc[:], in0=sc_sb[:, 0:1], in1=sc_sb[:, 1:2], op=OP.add
                )
                nc.vector.tensor_tensor(
                    out=sdcol[:, ti : ti + 1],
                    in0=sc_sb[:, 2:3],
                    in1=sc_sb[:, 3:4],
                    op=OP.add,
                )
                # transpose z back -> [P, fo]
                zr_ps = ptp.tile([P, fo], F32, tag="zr")
                nc.tensor.transpose(
                    out=zr_ps[:], in_=zT_sb[:, :], identity=ident_t[:fo, 0:fo]
                )
                zrow = pool.tile([P, fw], BF16, tag="zrow")
                nc.vector.tensor_copy(out=zrow[:, 0:fo], in_=zr_ps[:, :])
                nc.vector.memset(zrow[:, fo : fo + 1], 1.0)
                nc.vector.memset(zrow[:, fo + 3 : fw], 0.0)
                # s_hi / s_lo
                nc.vector.tensor_copy(out=zrow[:, fo + 1 : fo + 2], in_=ssrc[:, :])
                shi_f = pool.tile([P, 1], F32, tag="shif")
                nc.vector.tensor_copy(out=shi_f[:], in_=zrow[:, fo + 1 : fo + 2])
                nc.vector.tensor_tensor(
                    out=zrow[:, fo + 2 : fo + 3],
                    in0=ssrc[:, :],
                    in1=shi_f[:, :],
                    op=OP.subtract,
                )
                nc.sync.dma_start(
                    out=zloc[ti * P : (ti + 1) * P, :], in_=zrow[:, :]
                )

            # ================= Layer 1: ClusterGCN =================
            for ti in range(TILES_PER_CORE):
                    msg = gpool.tile([P, ku, FIN], BF16, tag="msg1")
                    for k in range(ku):
                        nc.gpsimd.indirect_dma_start(
                            out=msg[:, k, :],
                            out_offset=None,
                            in_=xtab[:, :],
                            in_offset=bass.IndirectOffsetOnAxis(
                                ap=midx_t[:, ti * ku + k : ti * ku + k + 1], axis=0
                            ),
                        )
                    sel = sel_build(ti)
                    acc = pacc.tile([P, FIN], F32, tag="acc")
                    for k in range(ku):
                        nc.tensor.matmul(
                            out=acc[:],
                            lhsT=sel[:, k, :],
                            rhs=msg[:, k, :],
                            start=(k == 0),
                            stop=(k == ku - 1),
                        )
                    # agg = deg_inv * acc  (f32)
                    agg = pool.tile([P, FIN], F32, tag="agg")
                    nc.vector.tensor_scalar(
                        out=agg[:],
                        in0=acc[:, :],
                        scalar1=dinv_t[:, ti : ti + 1],
                        scalar2=None,
                        op0=OP.mult,
                    )
                    # x_local tile
                    xl = pool.tile([P, FIN], F32, tag="xl")
                    nc.sync.dma_start(out=xl[:], in_=xloc[ti * P : (ti + 1) * P, :])
                    # transposes
                    aT_ps = ptp.tile([FIN, P], F32, tag="tp")
                    nc.tensor.transpose(out=aT_ps[:], in_=agg[:, :], identity=ident_t)
                    aT_sb = pool.tile([FIN, P], F32, tag="aT")
                    nc.vector.tensor_copy(out=aT_sb[:], in_=aT_ps[:])
                    xT_ps = ptp.tile([FIN, P], F32, tag="tp")
                    nc.tensor.transpose(out=xT_ps[:], in_=xl[:, :], identity=ident_t)
                    xT_sb = pool.tile([FIN, P], F32, tag="xT")
                    nc.vector.tensor_copy(out=xT_sb[:], in_=xT_ps[:])
                    # h1_T = Wout.T @ agg_T + Wroot.T @ x_T
                    hT_ps = ptp.tile([HID, P], F32, tag="zT")
                    nc.tensor.matmul(
                        out=hT_ps[:], lhsT=wout_t, rhs=aT_sb[:, :],
                        start=True, stop=False,
                    )
                    nc.tensor.matmul(
                        out=hT_ps[:], lhsT=wroot_t, rhs=xT_sb[:, :],
                        start=False, stop=True,
                    )
                    # relu(+bias) -> node-major via transpose path: keep f-major
                    h1T_sb = pool.tile([HID, P], F32, tag="h1T")
                    nc.scalar.activation(
                        out=h1T_sb[:], in_=hT_ps[:], func=AF.Relu, bias=bout_t
                    )
                    # back to node-major for the shared pack helper
                    h1_ps = ptp.tile([P, HID], F32, tag="zr")
                    nc.tensor.transpose(
                        out=h1_ps[:], in_=h1T_sb[:, :], identity=ident_t[:HID, 0:HID]
                    )
                    h1_sb = pool.tile([P, HID], F32, tag="h1")
                    nc.vector.tensor_copy(out=h1_sb[:], in_=h1_ps[:])
                    transform_and_pack(h1_sb, w1_t, a1_t, HID, HID, FW1, z1loc, ti)
            nc.sync.dma_start(
                out=sd1[:, :].rearrange("(p t) one -> p (t one)", p=P),
                in_=sdcol[:, :],
            )
            if phases >= 1:
                tc.strict_bb_all_engine_barrier()
                nc.gpsimd.collective_compute(
                    "AllGather",
                    OP.bypass,
                    replica_groups=groups,
                    ins=[z1loc[:, :]],
                    outs=[z1tab[:, :]],
                )
                tc.strict_bb_all_engine_barrier()

            # ================= Layers 2 & 3: GAT =================
            def gat_layer(ztab, sdt, fw, fo, w_t, a_t, brow_t, zloc_next, fw_next, sd_next, last):
                sdl = cpool.tile([P, TILES_PER_CORE], F32, tag=f"sdl{fw}")
                nc.sync.dma_start(
                    out=sdl[:],
                    in_=sdt[:, :].rearrange("(p t) one -> p (t one)", p=P),
                )
                # hi/lo bf16 split of s_dst for the bf16 expansion matmul
                sdlh = cpool.tile([P, TILES_PER_CORE, 2], BF16, tag=f"sdlh{fw}")
                nc.vector.tensor_copy(out=sdlh[:, :, 0], in_=sdl[:])
                hi_f = pool.tile([P, TILES_PER_CORE], F32, tag="hif")
                nc.vector.tensor_copy(out=hi_f[:], in_=sdlh[:, :, 0])
                nc.vector.tensor_tensor(
                    out=sdlh[:, :, 1], in0=sdl[:], in1=hi_f[:], op=OP.subtract
                )
                for ti in range(TILES_PER_CORE):
                        msg = gpool.tile([P, ku, fw], BF16, tag="msg2")
                        for k in range(ku):
                            nc.gpsimd.indirect_dma_start(
                                out=msg[:, k, :],
                                out_offset=None,
                                in_=ztab[:, :],
                                in_offset=bass.IndirectOffsetOnAxis(
                                    ap=midx_t[:, ti * ku + k : ti * ku + k + 1], axis=0
                                ),
                            )
                        sel = sel_build(ti)
                        sde2 = pool.tile([P, ku, 2], F32, tag="sde2")
                        for k in range(ku):
                            selT_ps = ptp.tile([P, P], BF16, tag="selT")
                            nc.tensor.transpose(
                                out=selT_ps[:], in_=sel[:, k, :], identity=identb_t
                            )
                            selT_sb = pool.tile([P, P], BF16, tag="selTsb")
                            nc.vector.tensor_copy(out=selT_sb[:], in_=selT_ps[:])
                            sde_ps = ptp.tile([P, 2], F32, tag="sdep")
                            nc.tensor.matmul(
                                out=sde_ps[:],
                                lhsT=selT_sb[:],
                                rhs=sdlh[:, ti, :],
                                start=True,
                                stop=True,
                            )
                            nc.vector.tensor_copy(out=sde2[:, k, :], in_=sde_ps[:])
                        o = 0
                        fz = fw - 4  # feature count in row
                        # logits l = s_src(hi+lo) + s_dst
                        l = pool.tile([P, ku], F32, tag="l")
                        nc.vector.tensor_tensor(
                            out=l[:],
                            in0=msg[:, o : o + ku, fz + 1],
                            in1=msg[:, o : o + ku, fz + 2],
                            op=OP.add,
                        )
                        nc.vector.tensor_tensor(
                            out=l[:], in0=l[:], in1=sde2[:, :, 0], op=OP.add
                        )
                        nc.vector.tensor_tensor(
                            out=l[:], in0=l[:], in1=sde2[:, :, 1], op=OP.add
                        )
                        lr = pool.tile([P, ku], F32, tag="lr")
                        nc.vector.tensor_scalar(
                            out=lr[:], in0=l[:], scalar1=NEG, scalar2=None, op0=OP.mult
                        )
                        nc.vector.tensor_tensor(out=lr[:], in0=l[:], in1=lr[:], op=OP.max)
                        w = pool.tile([P, ku], F32, tag="w")
                        nc.scalar.activation(out=w[:], in_=lr[:], func=AF.Exp)
                        wb = pool.tile([P, ku], BF16, tag="wb")
                        nc.vector.tensor_copy(out=wb[:], in_=w[:])
                        # weighted messages (+denominator column fz)
                        mp = pool.tile([P, ku, fz + 1], BF16, tag="mp")
                        nc.vector.tensor_tensor(
                            out=mp[:, :, :],
                            in0=msg[:, o : o + ku, 0 : fz + 1],
                            in1=wb[:, :, None].to_broadcast([P, ku, fz + 1]),
                            op=OP.mult,
                        )
                        acc = pacc.tile([P, fz + 1], F32, tag="acc")
                        for k in range(ku):
                            nc.tensor.matmul(
                                out=acc[:],
                                lhsT=sel[:, k, :],
                                rhs=mp[:, k, :],
                                start=(k == 0),
                                stop=(k == ku - 1),
                            )
                        den = pool.tile([P, 1], F32, tag="den")
                        nc.vector.tensor_scalar(
                            out=den[:], in0=acc[:, fz : fz + 1], scalar1=1e-30,
                            scalar2=None, op0=OP.max,
                        )
                        rec = pool.tile([P, 1], F32, tag="rec")
                        nc.vector.reciprocal(out=rec[:], in_=den[:])
                        h = pool.tile([P, fz], F32, tag="h")
                        nc.vector.tensor_scalar(
                            out=h[:], in0=acc[:, 0:fz], scalar1=rec[:, :],
                            scalar2=None, op0=OP.mult,
                        )
                        nc.vector.tensor_tensor(
                            out=h[:], in0=h[:], in1=brow_t, op=OP.add
                        )
                        if last:
                            nc.sync.dma_start(
                                out=outloc[ti * P : (ti + 1) * P, :], in_=h[:, :]
                            )
                        else:
                            nc.vector.tensor_scalar(
                                out=h[:], in0=h[:], scalar1=0.0, scalar2=None,
                                op0=OP.max,
                            )
                            transform_and_pack(h, w_t, a_t, fz, fw_next - 4, fw_next, zloc_next, ti)
                if not last:
                    nc.sync.dma_start(
                        out=sd_next[:, :].rearrange("(p t) one -> p (t one)", p=P),
                        in_=sdcol[:, :],
                    )

            if phases >= 2:
                gat_layer(z1tab, sd1, FW1, HID, w2_t, a2_t, b1r_t, z2loc, FW2, sd2, False)
            if phases >= 3:
                tc.strict_bb_all_engine_barrier()
                nc.gpsimd.collective_compute(
                    "AllGather",
                    OP.bypass,
                    replica_groups=groups,
                    ins=[z2loc[:, :]],
                    outs=[z2tab[:, :]],
                )
                tc.strict_bb_all_engine_barrier()
                gat_layer(z2tab, sd2, FW2, FOUT, None, None, b2r_t, None, None, None, True)
            if phases < 3:
                # dummy write so outloc is produced
                for ti in range(TILES_PER_CORE):
                    zt = pool.tile([P, FOUT], F32, tag="h")
                    nc.vector.memset(zt[:], 0.0)
                    nc.sync.dma_start(out=outloc[ti * P : (ti + 1) * P, :], in_=zt[:, :])

    nc.finalize()
    return nc


# ----------------------------------------------------------------------------
# entry point
# ----------------------------------------------------------------------------
def kernel(
    x,
    edge_index,
    W_out,
    b_out,
    W_root,
    W1,
    a_src1,
    a_dst1,
    b1,
    W2,
    a_src2,
    a_dst2,
    b2,
    training=0,
    **_unused,
):
    pre = _preprocess(x, edge_index)
    ku = pre["ku"]
    import os as _os
    _key = (ku, _os.environ.get("KERNEL_PHASES", "3"))
    if _key not in _cache:
        _cache[_key] = _build_program(ku)
    nc = _cache[_key]

    iota = np.tile(np.arange(P, dtype=np.float32), (P, 1))
    ident = np.eye(P, dtype=np.float32)

    a1hi, a1lo = _hilo(np.asarray(a_src1, np.float32))
    a1dhi, a1dlo = _hilo(np.asarray(a_dst1, np.float32))
    a2hi, a2lo = _hilo(np.asarray(a_src2, np.float32))
    a2dhi, a2dlo = _hilo(np.asarray(a_dst2, np.float32))
    a1 = np.stack(
        [np.float32(a1hi), np.float32(a1lo), np.float32(a1dhi), np.float32(a1dlo)], 1
    )
    a2 = np.stack(
        [np.float32(a2hi), np.float32(a2lo), np.float32(a2dhi), np.float32(a2dlo)], 1
    )

    xp = pre["xp"]
    in_maps = []
    for c in range(NCORES):
        in_maps.append(
            {
                "xtab": _bf16(xp),
                "xloc": np.ascontiguousarray(xp[c * NPC : (c + 1) * NPC]),
                "constf": np.concatenate(
                    [
                        ident,
                        pre["deginv"][c],
                        np.tile(np.asarray(b1, np.float32), (P, 1)),
                        np.tile(np.asarray(b2, np.float32), (P, 1)),
                        _padP(np.asarray(b_out, np.float32).reshape(HID, 1)),
                        _padP(a1),
                        _padP(a2),
                        _padP(np.asarray(W_out, np.float32)),
                        _padP(np.asarray(W_root, np.float32)),
                        _padP(np.asarray(W1, np.float32)),
                        _padP(np.asarray(W2, np.float32)),
                    ],
                    axis=1,
                ),
                "constb": np.concatenate(
                    [_bf16(iota), _bf16(pre["mloc"][c]), _bf16(ident)], axis=1
                ),
                "consti": np.concatenate(
                    [pre["midx"][c], pre["mdst"][c]], axis=1
                ),
            }
        )

    import os
    trace = bool(os.environ.get("BASS_TRACE"))
    res = run_bass_kernel_spmd(
        nc, in_maps, list(range(NCORES)), trace=trace
    )
    global last_result
    last_result = res
    out_p = np.concatenate([res.results[c]["outloc"] for c in range(NCORES)], 0)
    out = out_p[pre["gid"][:N]]
    return np.asarray(out, np.float32)

